# revision 1
# baseline (speedup 1.0000x reference)
"""Trainium2 Bass kernel for nn_CondRnnSampler — v2 (fp8 DoubleRow + all-tanh).

Per-core (512 rows), per step:
  MLP:   hid = relu(W1 h), logits = W2 hid, e = exp(logits), prod = logits*oh
  cell:  gates = W_ih x + W_hh h (fp8 DoubleRow, K=256/instr)
         all-sigmoid rewritten as tanh via sigma(z) = (1+tanh(z/2))/2 with the
         1/2 folded into weight rows, so every gate activation is a plain tanh
         and the in-loop ACT table set is {tanh, exp} (exp_and_others) — no
         table switching.  State: s = 2c (bf16), v = 2h (fp8):
           s' = 0.5*(1+tf)*s + (1+ti)*g ;  v' = (1+to)*tanh(0.5 s')
  out:   esum/pick accumulate into one PSUM bank (rows 0-63 esum, 64-127 pick)
         via sliding-selector fp8 DoubleRow matmuls.

Scales (folded on host): x8 = 64*x, v = 2h, hid8 = 8*hid, gates PSUM = beta*a,
logits PSUM = delta*l.  One-hots (sample) and gathered positional encodings
are built host-side and DMA-streamed per step.
"""

import sys

sys.path.insert(0, "/opt/trn_rl_repo")

from contextlib import ExitStack

import ml_dtypes
import numpy as np

import concourse.bacc as bacc
import concourse.tile as tile
from concourse import bass_utils, mybir
from concourse.bass import ts

B, D, E, NCL = 4096, 64, 256, 256
NCORES = 8
BS = B // NCORES
P = 128

AF = mybir.ActivationFunctionType
OP = mybir.AluOpType
F32 = mybir.dt.float32
BF16 = mybir.dt.bfloat16
FP8 = mybir.dt.float8e4
DR = mybir.MatmulPerfMode.DoubleRow
NPBF = ml_dtypes.bfloat16
NPF8 = ml_dtypes.float8_e4m3

SX = 64.0  # x fp8 scale
SH = 8.0  # hid fp8 scale
DELTA = 256.0  # logits PSUM scale


def _pe_table() -> np.ndarray:
    half = np.float32(E // 2)
    inv = (
        np.float32(1.0)
        / (np.float32(10000.0) ** (np.arange(E // 2, dtype=np.float32) / half))
    ).astype(np.float32)
    pos = np.arange(D, dtype=np.float32)[:, None]
    ang = pos * inv[None, :]
    return np.concatenate([np.sin(ang), np.cos(ang)], axis=1).astype(np.float32)


def _q8(x):
    return np.clip(np.asarray(x, np.float32), -240, 240).astype(NPF8)


def build_bass(n_steps: int = D):
    nc = bacc.Bacc("TRN2", debug=False, target_bir_lowering=False, num_devices=NCORES)

    def din(name, shape, dt):
        return nc.dram_tensor(name, list(shape), dt, kind="ExternalInput").ap()

    wih_d = din("wih", (P, 2, 4 * E), FP8)
    whh_d = din("whh", (P, 2, 4 * E), FP8)
    w1_d = din("w1", (P, 2, 2 * E), FP8)
    w2_d = din("w2", (P, 4, NCL), FP8)
    te_d = din("te", (P, 2, E), FP8)
    slide_d = din("slide", (P, 2, 2 * D), FP8)  # ones at col D-1 (both halves)
    ones64_d = din("ones64", (D, 1), F32)
    ohs_d = din("ohs", (D, P, 2, BS), FP8)  # one-hot(sample) per step
    xpe_d = din("xpe", (D, P, 2, BS), FP8)  # 64*petab[pos] per step
    out_d = nc.dram_tensor("out", [1, BS], F32, kind="ExternalOutput").ap()

    with tile.TileContext(nc) as tc:
        with ExitStack() as ctx:
            sing = ctx.enter_context(tc.tile_pool(name="sing", bufs=1))
            gt = ctx.enter_context(tc.tile_pool(name="gt", bufs=6))
            xp = ctx.enter_context(tc.tile_pool(name="xp", bufs=3))
            hp = ctx.enter_context(tc.tile_pool(name="hp", bufs=3))
            ep = ctx.enter_context(tc.tile_pool(name="ep", bufs=4))
            lp = ctx.enter_context(tc.tile_pool(name="lp", bufs=2))
            psing = ctx.enter_context(tc.tile_pool(name="psing", bufs=1, space="PSUM"))
            pp = ctx.enter_context(tc.tile_pool(name="pp", bufs=3, space="PSUM"))

            # ---- resident tensors -------------------------------------
            # init-critical first: step-0 one-hot/pe slices + gate weights
            ohs_sb = sing.tile([P, D, 2, BS], FP8, tag="ohs")
            xpe_sb = sing.tile([P, D, 2, BS], FP8, tag="xpe")
            nc.sync.dma_start(xpe_sb[:, 0], xpe_d[0])
            nc.sync.dma_start(ohs_sb[:, 0], ohs_d[0])
            wih = sing.tile([P, 2, 4 * E], FP8, tag="wih")
            nc.sync.dma_start(wih[:], wih_d)
            whh = sing.tile([P, 2, 4 * E], FP8, tag="whh")
            nc.sync.dma_start(whh[:], whh_d)
            w1 = sing.tile([P, 2, 2 * E], FP8, tag="w1")
            nc.sync.dma_start(w1[:], w1_d)
            w2 = sing.tile([P, 4, NCL], FP8, tag="w2")
            nc.sync.dma_start(w2[:], w2_d)
            te = sing.tile([P, 2, E], FP8, tag="te")
            nc.sync.dma_start(te[:], te_d)
            slide = sing.tile([P, 2, 2 * D], FP8, tag="slide")
            nc.sync.dma_start(slide[:], slide_d)
            ones64 = sing.tile([D, 1], F32, tag="ones64")
            nc.sync.dma_start(ones64[:], ones64_d)

            for i in range(1, n_steps):
                nc.sync.dma_start(ohs_sb[:, i], ohs_d[i])
                nc.sync.dma_start(xpe_sb[:, i], xpe_d[i])

            # double-buffered recurrent state (parity by step)
            s_bufs = [
                sing.tile([P, 2, BS], BF16, tag=f"s{j}", name=f"s{j}")
                for j in range(2)
            ]
            v_bufs = [
                sing.tile([P, 2, BS], FP8, tag=f"v{j}", name=f"v{j}")
                for j in range(2)
            ]
            T_sb = sing.tile([P, 2, BS], BF16, tag="T")
            esum_ps = psing.tile([D, BS], F32, tag="esum")
            pick_ps = psing.tile([D, BS], F32, tag="pick")

            # scales arrive via sc tile? No - bake as python floats at build:
            # (they depend only on weight maxima; recomputed per call would
            # need rebuild. Instead scales are fixed: beta/gamma baked by
            # prep_inputs to match BETA/GAMMA globals.)

            def gate_step(x8_ap, v_prev, with_h, inv_beta):
                """gates -> t tiles [ti, tf, g, to]; order f,g,i,o so the
                chain ops X1 (needs tf) and X2 (needs g) unblock earliest."""
                tg = [None] * 4
                # v-independent wih matmuls for the chain-leading f/g gates
                # are emitted at normal priority AHEAD of any whh matmul, so
                # the in-order PE queue runs them during the v-wait bubble
                # instead of stalling behind the first v-dependent whh.
                pre = {}
                if with_h:
                    for gi in (1, 2, 0):  # f, g, i
                        g_ps = pp.tile([P, 2, BS], F32, tag="ps")
                        for k in range(2):
                            nc.tensor.matmul(
                                g_ps[:, k, :], wih[:, :, ts(gi * 2 + k, P)],
                                x8_ap, start=True, stop=False, perf_mode=DR,
                            )
                        pre[gi] = g_ps
                with tc.high_priority():
                    for gi in (1, 2, 0, 3):  # f, g, i, o
                        if gi in pre:
                            g_ps = pre[gi]
                            for k in range(2):
                                nc.tensor.matmul(
                                    g_ps[:, k, :], whh[:, :, ts(gi * 2 + k, P)],
                                    v_prev[:], start=False, stop=True,
                                    perf_mode=DR,
                                )
                        else:
                            g_ps = pp.tile([P, 2, BS], F32, tag="ps")
                            for k in range(2):
                                m = gi * 2 + k
                                nc.tensor.matmul(
                                    g_ps[:, k, :], wih[:, :, ts(m, P)], x8_ap,
                                    start=True, stop=not with_h, perf_mode=DR,
                                )
                                if with_h:
                                    nc.tensor.matmul(
                                        g_ps[:, k, :], whh[:, :, ts(m, P)],
                                        v_prev[:], start=False, stop=True,
                                        perf_mode=DR,
                                    )
                        t_sb = gt.tile([P, 2, BS], BF16, tag="t")
                        nc.scalar.activation(
                            t_sb[:], g_ps[:], AF.Tanh, scale=inv_beta
                        )
                        tg[gi] = t_sb
                return tg

            def tail(tg, s_prev, s_cur, v_cur, first):
                """Recurrent-chain ops at high priority so the scheduler's
                static per-engine orders never park bulk work (relu/prod/
                x-add/exp) in front of them."""
                ti, tf, g, to = tg[0], tg[1], tg[2], tg[3]
                with tc.high_priority():
                    if first:
                        # s = (1+ti)*g
                        nc.vector.scalar_tensor_tensor(
                            s_cur[:], ti[:], 1.0, g[:], OP.add, OP.mult
                        )
                    else:
                        x1 = gt.tile([P, 2, BS], BF16, tag="x1")
                        nc.vector.scalar_tensor_tensor(
                            x1[:], tf[:], 1.0, s_prev[:], OP.add, OP.mult
                        )
                        x2 = gt.tile([P, 2, BS], BF16, tag="x2")
                        nc.vector.scalar_tensor_tensor(
                            x2[:], ti[:], 1.0, g[:], OP.add, OP.mult
                        )
                        nc.vector.scalar_tensor_tensor(
                            s_cur[:], x1[:], 0.5, x2[:], OP.mult, OP.add
                        )
                    nc.scalar.activation(T_sb[:], s_cur[:], AF.Tanh, scale=0.5)
                    nc.vector.scalar_tensor_tensor(
                        v_cur[:], to[:], 1.0, T_sb[:], OP.add, OP.mult
                    )

            inv_beta = float(1.0 / _BETA)
            hid_scale = float(SH / _GAMMA)
            inv_delta = float(1.0 / DELTA)

            # ---- init: lstm(pe[:,0]) with zero state ------------------
            # init state lands in parity-1 buffers (step 0 reads [1],
            # writes [0]; step i reads [i%2^1]... step i writes [i%2]).
            tg0 = gate_step(xpe_sb[:, 0], None, with_h=False, inv_beta=inv_beta)
            tail(tg0, None, s_bufs[1], v_bufs[1], first=True)

            # x8 for step 0: te[s_0] + pe_0
            def build_x(i):
                x_ps = pp.tile([P, 2, BS], F32, tag="ps")
                for t in range(2):
                    nc.tensor.matmul(
                        x_ps[:, t, :], te[:, :, ts(t, P)], ohs_sb[:, i],
                        start=True, stop=True, perf_mode=DR,
                    )
                x8 = xp.tile([P, 2, BS], FP8, tag="x8")
                for k in range(2):
                    nc.vector.tensor_tensor(
                        x8[:, k, :], x_ps[:, k, :], xpe_sb[:, i, k, :], OP.add
                    )
                return x8

            x8_t = {0: build_x(0)}

            pending = []  # deferred (step, e8, pr8) awaiting esum/pick MMs

            def flush_accum(j, e8_j, pr8_j):
                for k in range(2):
                    nc.tensor.matmul(
                        esum_ps[:], slide[:, k, D - 1 - j : 2 * D - 1 - j],
                        e8_j[:, k, :], start=(j == 0 and k == 0),
                        stop=(j == n_steps - 1 and k == 1),
                        skip_group_check=True,
                    )
                    nc.tensor.matmul(
                        pick_ps[:], slide[:, k, D - 1 - j : 2 * D - 1 - j],
                        pr8_j[:, k, :], start=(j == 0 and k == 0),
                        stop=(j == n_steps - 1 and k == 1),
                        skip_group_check=True,
                    )

            # ---- scan -------------------------------------------------
            for i in range(n_steps):
                v_prev, v_cur = v_bufs[(i + 1) % 2], v_bufs[i % 2]
                s_prev, s_cur = s_bufs[(i + 1) % 2], s_bufs[i % 2]

                # gates + cell update FIRST (the serial chain)
                tg = gate_step(
                    x8_t.pop(i)[:], v_prev, with_h=True, inv_beta=inv_beta
                )
                tail(tg, s_prev, s_cur, v_cur, first=False)

                # MLP from v_{i-1} (h-ready at step start; fills PE bubbles)
                hid8 = []
                for hh in range(2):
                    h_ps = pp.tile([P, 2, BS], F32, tag="ps")
                    for k in range(2):
                        m = hh * 2 + k
                        nc.tensor.matmul(
                            h_ps[:, k, :], w1[:, :, ts(m, P)], v_prev[:],
                            start=True, stop=True, perf_mode=DR,
                        )
                    h8 = hp.tile([P, 2, BS], FP8, tag="h8")
                    # relu on ACT: frees the hid PSUM banks right away and
                    # unblocks the W2 matmuls without queueing behind the
                    # chain-deprioritized DVE ops
                    nc.scalar.activation(h8[:], h_ps[:], AF.Relu, scale=hid_scale)
                    hid8.append(h8)
                l_ps = pp.tile([P, 2, BS], F32, tag="ps")
                for t in range(2):
                    for j in range(2):
                        nc.tensor.matmul(
                            l_ps[:, t, :], w2[:, 2 * j : 2 * j + 2, ts(t, P)],
                            hid8[j][:], start=(j == 0), stop=(j == 1),
                            perf_mode=DR,
                        )
                e8 = ep.tile([P, 2, BS], FP8, tag="e8")
                nc.scalar.activation(e8[:], l_ps[:], AF.Exp, scale=inv_delta)
                # stash logits to SBUF so the PSUM banks free after the two
                # ACT reads instead of waiting for the (chain-deprioritized)
                # DVE prod ops late in the step
                l_bf = lp.tile([P, 2, BS], BF16, tag="lbf")
                nc.scalar.activation(l_bf[:], l_ps[:], AF.Copy)
                pr8 = ep.tile([P, 2, BS], FP8, tag="pr8")
                for k in range(2):
                    nc.vector.tensor_tensor(
                        pr8[:, k, :], l_bf[:, k, :], ohs_sb[:, i, k, :], OP.mult
                    )

                if i + 1 < n_steps:
                    x8_t[i + 1] = build_x(i + 1)

                # esum/pick accumulation (fp8 non-DR; M=64 dst), deferred by
                # one step so these MMs never sit in the PE's in-order queue
                # ahead of the next step's chain-critical gate matmuls while
                # still waiting on exp/prod outputs.
                pending.append((i, e8, pr8))
                if i > 0:
                    flush_accum(*pending.pop(0))

            # ---- epilogue ---------------------------------------------
            while pending:
                flush_accum(*pending.pop(0))
            ln_e = sing.tile([D, BS], F32, tag="lne")
            nc.scalar.activation(ln_e[:], esum_ps[:], AF.Ln)
            diff = sing.tile([D, BS], F32, tag="diff")
            nc.vector.scalar_tensor_tensor(
                diff[:], pick_ps[:], inv_delta, ln_e[:],
                OP.mult, OP.subtract,
            )
            fin_ps = pp.tile([P, 2, BS], F32, tag="ps")
            nc.tensor.matmul(
                fin_ps[0:1, 0, :], ones64[:, 0:1], diff[:], start=True, stop=True
            )
            out_sb = sing.tile([1, BS], F32, tag="outsb")
            nc.scalar.activation(out_sb[:], fin_ps[0:1, 0, :], AF.Copy)
            nc.sync.dma_start(out_d, out_sb[:])

    nc.compile()
    return nc


_BETA = None
_GAMMA = None


def _compute_scales(W_ih, W_hh, W1):
    half = np.ones((4 * E, 1), np.float32)
    half[: 2 * E] = 0.5
    half[3 * E :] = 0.5
    Wg_ih = np.asarray(W_ih, np.float32) * half
    Wg_hh = np.asarray(W_hh, np.float32) * half
    beta = 216.0 / max(np.abs(Wg_ih / SX).max(), np.abs(Wg_hh / 2.0).max())
    gamma = 216.0 / np.abs(np.asarray(W1, np.float32) / 2.0).max()
    return beta, gamma, Wg_ih, Wg_hh


def prep_inputs(token_embed, W_ih, b_ih, b_hh, W_hh, W1, b1, W2, b2, pos_list,
                input_samples):
    f = np.float32
    for b in (b_ih, b_hh, b1, b2):
        assert np.all(np.asarray(b) == 0), "nonzero biases unsupported"
    beta, gamma, Wg_ih, Wg_hh = _compute_scales(W_ih, W_hh, W1)
    assert beta == _BETA and gamma == _GAMMA

    def lhsT8(Wt, ko):  # [K, M] -> [P, ko, M] fp8
        K, M = Wt.shape
        return np.ascontiguousarray(
            _q8(Wt).reshape(ko, P, M).transpose(1, 0, 2)
        )

    petab = _pe_table()
    slide = np.zeros((P, 2, 2 * D), f)
    slide[:, :, D - 1] = 1.0

    shared = {
        "wih": lhsT8(beta / SX * Wg_ih.T, 2),
        "whh": lhsT8(beta / 2.0 * Wg_hh.T, 2),
        "w1": lhsT8(gamma / 2.0 * np.asarray(W1, f).T, 2),
        "w2": lhsT8(DELTA / SH * np.asarray(W2, f).T, 4),
        "te": lhsT8(SX * np.asarray(token_embed, f), 2),
        "slide": _q8(slide),
        "ones64": np.ones((D, 1), f),
    }
    samples = np.asarray(input_samples)
    poss = np.asarray(pos_list)
    pe8 = _q8(SX * petab)  # [D, E] fp8 rows
    in_maps = []
    for c in range(NCORES):
        lo, hi = c * BS, (c + 1) * BS
        sa = samples[lo:hi]  # [BS, D]
        po = poss[lo:hi]
        ohs = np.zeros((D, 2, P, BS), NPF8)
        ii = np.arange(BS)
        for i in range(D):
            s = np.asarray(sa[:, i])
            ohs[i, s // P, s % P, ii] = 1.0
        ohs = np.ascontiguousarray(ohs.transpose(0, 2, 1, 3))
        xpe = pe8[po.T]  # [D, BS, E]
        xpe = np.ascontiguousarray(
            xpe.transpose(0, 2, 1).reshape(D, 2, P, BS).transpose(0, 2, 1, 3)
        )
        m = dict(shared)
        m["ohs"] = ohs
        m["xpe"] = xpe
        in_maps.append(m)
    return in_maps


_CACHE = {}


def kernel(**inputs) -> np.ndarray:
    global _BETA, _GAMMA
    if "nc" not in _CACHE:
        _BETA, _GAMMA, _, _ = _compute_scales(
            inputs["W_ih"], inputs["W_hh"], inputs["W1"]
        )
        _CACHE["nc"] = build_bass()
    nc = _CACHE["nc"]
    in_maps = prep_inputs(**inputs)
    res = bass_utils.run_bass_kernel_spmd(nc, in_maps, core_ids=list(range(NCORES)))
    _CACHE["last_results"] = res
    out = np.empty((B, 1), np.float32)
    for c in range(NCORES):
        out[c * BS : (c + 1) * BS, 0] = np.asarray(
            res.results[c]["out"], np.float32
        ).reshape(BS)
    return out

